# revision 1
# baseline (speedup 1.0000x reference)
"""Multi-head attention (B=2, C=2048, M=1024, H=16, K=V=64) on 8 TRN2 NeuronCores.

Sharding: core i handles batch b = i//4 and query slice s = i%4 (512 queries).
Each core computes K/V projections for all heads over all 2048 keys (duplicated
within each 4-core batch group), Q projection for its 512 queries, attention
with partition-axis softmax (keys on partitions, queries on free axis), and the
full output projection for its query slice. No inter-core communication.

All matmuls run in bf16 with fp32 PSUM accumulation. The softmax denominator is
obtained by augmenting the V matrix with a ones column so the att@V matmul also
produces sum_c(att) for free; normalization happens on the small [64, 512] "pre"
tensor before the output projection.
"""

import os
import sys

import numpy as np
import ml_dtypes

for _p in ("/opt/trn_rl_repo", os.path.expanduser("~/.axon_site/_ro/trn_rl_repo")):
    if os.path.isdir(_p) and _p not in sys.path:
        sys.path.insert(0, _p)

import concourse.bacc as bacc
import concourse.tile as tile
from concourse import mybir
from concourse.bass_utils import run_bass_kernel_spmd

B, C, M, H, KD, VD = 2, 2048, 1024, 16, 64, 64
N_CORES = 8
DQ = 512  # queries per core
INV_SCALE = 1.0 / 8.0  # 1/sqrt(KD)
BF16 = mybir.dt.bfloat16
F32 = mybir.dt.float32
NPBF16 = ml_dtypes.bfloat16

NJ = M // 128  # 8 m-chunks
NP = H // 2  # 8 head pairs
NCC = C // 128  # 16 key chunks
ND = DQ // 128  # 4 query sub-chunks


def _build():
    nc = bacc.Bacc("TRN2", target_bir_lowering=False, debug=False, num_devices=N_CORES)

    xq_d = nc.dram_tensor("xq", [128, NJ, DQ], BF16, kind="ExternalInput").ap()
    xkv_d = nc.dram_tensor("xkv", [128, NJ, C], BF16, kind="ExternalInput").ap()
    wqp_d = nc.dram_tensor("wqp", [128, NP, NJ, 128], BF16, kind="ExternalInput").ap()
    wkp_d = nc.dram_tensor("wkp", [128, NP, NJ, 128], BF16, kind="ExternalInput").ap()
    wv_d = nc.dram_tensor("wv", [128, NJ, H * VD], BF16, kind="ExternalInput").ap()
    woa_d = nc.dram_tensor("woa", [64, 8, M], BF16, kind="ExternalInput").ap()
    wob_d = nc.dram_tensor("wob", [64, 8, M], BF16, kind="ExternalInput").ap()
    out_d = nc.dram_tensor("out", [DQ, M], F32, kind="ExternalOutput").ap()

    with tile.TileContext(nc) as tc:
        with (
            tc.tile_pool(name="sb", bufs=1) as sb,
            tc.tile_pool(name="ps", space="PSUM", bufs=1) as ps,
        ):
            # ---- input loads -------------------------------------------------
            wkp_sb = sb.tile([128, NP, NJ, 128], BF16, tag="wp", bufs=2)
            nc.sync.dma_start(out=wkp_sb, in_=wkp_d)
            wqp_sb = sb.tile([128, NP, NJ, 128], BF16, tag="wp", bufs=2)
            nc.sync.dma_start(out=wqp_sb, in_=wqp_d)
            xkv_sb = sb.tile([128, NJ, C], BF16)
            for j in range(NJ):
                nc.sync.dma_start(out=xkv_sb[:, j, :], in_=xkv_d[:, j, :])
            xq_sb = sb.tile([128, NJ, DQ], BF16, tag="xq")
            for j in range(NJ):
                nc.sync.dma_start(out=xq_sb[:, j, :], in_=xq_d[:, j, :])
            wv_sb = sb.tile([128, NJ, H * VD], BF16, tag="wv")
            nc.sync.dma_start(out=wv_sb, in_=wv_d)

            # ---- K^T projection: kt[k2, p, c] for head pair p ---------------
            kt_sb = sb.tile([128, NP, C], BF16)
            for p in range(NP):
                for c4 in range(C // 512):
                    pk = ps.tile([128, 512], F32, tag="pk", bufs=2, name=f"pk_k{p}_{c4}")
                    for j in range(NJ):
                        nc.tensor.matmul(
                            pk,
                            wkp_sb[:, p, j, :],
                            xkv_sb[:, j, c4 * 512 : (c4 + 1) * 512],
                            start=(j == 0),
                            stop=(j == NJ - 1),
                        )
                    nc.vector.tensor_copy(kt_sb[:, p, c4 * 512 : (c4 + 1) * 512], pk)

            # ---- Q^T projection: qt[k2, p, d] -------------------------------
            qt_sb = sb.tile([128, NP, DQ], BF16)
            for p in range(NP):
                pk = ps.tile([128, 512], F32, tag="pk", bufs=2, name=f"pk_q{p}")
                for j in range(NJ):
                    nc.tensor.matmul(
                        pk,
                        wqp_sb[:, p, j, :],
                        xq_sb[:, j, :],
                        start=(j == 0),
                        stop=(j == NJ - 1),
                    )
                nc.vector.tensor_copy(qt_sb[:, p, :], pk)

            # ---- V projection (+ ones column): v[c_sub, cc, h, 65] ----------
            v_sb = sb.tile([128, NCC, H, VD + 1], BF16)
            nc.vector.memset(v_sb[:, :, :, VD : VD + 1], 1.0)
            for cc in range(NCC):
                for half in range(2):
                    pv = ps.tile(
                        [128, 512], F32, tag="pbig", bufs=2, name=f"pv_{cc}_{half}"
                    )
                    for j in range(NJ):
                        nc.tensor.matmul(
                            pv,
                            xkv_sb[:, j, cc * 128 : (cc + 1) * 128],
                            wv_sb[:, j, half * 512 : (half + 1) * 512],
                            start=(j == 0),
                            stop=(j == NJ - 1),
                        )
                    nc.vector.tensor_copy(
                        v_sb[:, cc, half * 8 : (half + 1) * 8, 0:VD],
                        pv.rearrange("q (h v) -> q h v", h=8),
                    )

            # ---- attention: per head, keys on partitions --------------------
            pre_sb = sb.tile([64, H, DQ], BF16, tag="xq")
            for h in range(H):
                p, r = divmod(h, 2)
                pp = ps.tile([65, 512], F32, tag="pre", bufs=2, name=f"pre_{h}")
                for cc in range(NCC):
                    lg = ps.tile(
                        [128, 512], F32, tag="pk", bufs=2, name=f"lg_{h}_{cc}"
                    )
                    nc.tensor.matmul(
                        lg,
                        kt_sb[64 * r : 64 * (r + 1), p, cc * 128 : (cc + 1) * 128],
                        qt_sb[64 * r : 64 * (r + 1), p, :],
                        start=True,
                        stop=True,
                    )
                    att = sb.tile([128, 512], BF16, tag="att", bufs=4, name=f"att_{h}_{cc}")
                    nc.scalar.activation(
                        att, lg, mybir.ActivationFunctionType.Exp, scale=INV_SCALE
                    )
                    nc.tensor.matmul(
                        pp,
                        v_sb[:, cc, h, :],
                        att,
                        start=(cc == 0),
                        stop=(cc == NCC - 1),
                    )
                rec = sb.tile([1, 512], F32, tag="rec", bufs=2, name=f"rec_{h}")
                nc.vector.reciprocal(rec, pp[64:65, :])
                recb = sb.tile([64, 512], F32, tag="recb", bufs=2, name=f"recb_{h}")
                nc.gpsimd.partition_broadcast(recb, rec)
                nc.vector.tensor_mul(pre_sb[:, h, :], pp[0:64, :], recb)

            # ---- output projection: out[d, m] = sum_h pre[v, h, d].T @ wo ---
            wo_sb = [
                sb.tile([64, 8, M], BF16, tag="wp", bufs=2, name="woa_sb"),
                sb.tile([64, 8, M], BF16, tag="wv", name="wob_sb"),
            ]
            nc.sync.dma_start(out=wo_sb[0], in_=woa_d)
            nc.sync.dma_start(out=wo_sb[1], in_=wob_d)
            for ds in range(ND):
                po = [
                    ps.tile([128, 512], F32, tag="pbig", bufs=2, name=f"po_{ds}_0"),
                    ps.tile([128, 512], F32, tag="pbig", bufs=2, name=f"po_{ds}_1"),
                ]
                for h in range(H):
                    for half in range(2):
                        nc.tensor.matmul(
                            po[half],
                            pre_sb[:, h, ds * 128 : (ds + 1) * 128],
                            wo_sb[h // 8][:, h % 8, half * 512 : (half + 1) * 512],
                            start=(h == 0),
                            stop=(h == H - 1),
                        )
                osb = sb.tile([128, M], F32, tag="osb", bufs=2, name=f"osb_{ds}")
                nc.vector.tensor_copy(osb[:, 0:512], po[0])
                nc.vector.tensor_copy(osb[:, 512:1024], po[1])
                nc.sync.dma_start(
                    out=out_d[ds * 128 : (ds + 1) * 128, :], in_=osb
                )

    nc.compile()
    return nc


_CACHE: dict = {}


def _prep_inputs(kvinput, qinput, wq, wk, wv, wo):
    """Host-side sharding/layout prep. Returns per-core input dicts."""

    def bf16c(a):
        return np.ascontiguousarray(a.astype(NPBF16))

    def pack_pairs(w):  # [H, M, 64] -> [128(q), NP, NJ, 128]
        a = w.reshape(NP, 2, M, KD).transpose(0, 2, 1, 3).reshape(NP, M, 128)
        return bf16c(a.reshape(NP, NJ, 128, 128).transpose(2, 0, 1, 3))

    wqp = pack_pairs(np.asarray(wq, np.float32))
    wkp = pack_pairs(np.asarray(wk, np.float32))
    wvh = bf16c(
        np.asarray(wv, np.float32)
        .transpose(1, 0, 2)
        .reshape(NJ, 128, H * VD)
        .transpose(1, 0, 2)
    )
    wof = np.asarray(wo, np.float32).transpose(1, 0, 2)  # [64, H, M]
    woa = bf16c(wof[:, 0:8, :])
    wob = bf16c(wof[:, 8:16, :])

    kvinput = np.asarray(kvinput, np.float32)
    qinput = np.asarray(qinput, np.float32)
    xkv_b = [
        bf16c(kvinput[b].T.reshape(NJ, 128, C).transpose(1, 0, 2)) for b in range(B)
    ]

    in_maps = []
    for core in range(N_CORES):
        b, s = divmod(core, 4)
        xq = bf16c(
            qinput[b, s * DQ : (s + 1) * DQ, :].T.reshape(NJ, 128, DQ).transpose(1, 0, 2)
        )
        in_maps.append(
            {
                "xq": xq,
                "xkv": xkv_b[b],
                "wqp": wqp,
                "wkp": wkp,
                "wv": wvh,
                "woa": woa,
                "wob": wob,
            }
        )
    return in_maps


def kernel(**inputs):
    if "nc" not in _CACHE:
        _CACHE["nc"] = _build()
    nc = _CACHE["nc"]
    in_maps = _prep_inputs(
        inputs["kvinput"],
        inputs["qinput"],
        inputs["wq"],
        inputs["wk"],
        inputs["wv"],
        inputs["wo"],
    )
    _CACHE["in_maps"] = in_maps
    res = run_bass_kernel_spmd(nc, in_maps, core_ids=list(range(N_CORES)))
    out = np.empty((B, C, M), np.float32)
    for core in range(N_CORES):
        b, s = divmod(core, 4)
        out[b, s * DQ : (s + 1) * DQ, :] = res.results[core]["out"]
    return out


# revision 13
# speedup vs baseline: 1.0864x; 1.0864x over previous
"""Multi-head attention (B=2, C=2048, M=1024, H=16, K=V=64) on 8 TRN2 NeuronCores.

Sharding (per the tensor-parallel hint, with a cheaper exchange): core i handles
batch b = i//4 and heads 4r..4r+3 where r = i%4. Each core projects K/V/Q for
its 4 heads, runs attention for those heads over all 2048 queries, then an
AllToAll within each 4-core batch group redistributes the normalized attention
output "pre" [v, h, d] (1 MB bf16 per core) from head-sharded to query-sharded,
so every core computes the full output projection for its 512-query slice with
no reduction.

All matmuls are bf16 with fp32 PSUM accumulation. Softmax runs along the PSUM
partition axis (keys on partitions): exp via ScalarE on wide [128, 1024] tiles,
denominator via a ones-column folded into the att@V matmul, normalization via
fast approximate reciprocal + gpsimd partition-broadcast on the small pre
tensor. No max-subtraction is needed: scaled logits are ~N(0,1) so exp stays
in fp32 range.
"""

import os
import sys

import numpy as np
import ml_dtypes

for _p in ("/opt/trn_rl_repo", os.path.expanduser("~/.axon_site/_ro/trn_rl_repo")):
    if os.path.isdir(_p) and _p not in sys.path:
        sys.path.insert(0, _p)

import concourse.bacc as bacc
import concourse.tile as tile
from concourse import mybir
from concourse.bass_utils import run_bass_kernel_spmd

B, C, M, H, KD, VD = 2, 2048, 1024, 16, 64, 64
N_CORES = 8
GROUP = 4  # cores per batch group
HL = H // GROUP  # 4 local heads per core
DQ = C // GROUP  # 512-query output slice per core
INV_SCALE = 1.0 / 8.0  # 1/sqrt(KD)
BF16 = mybir.dt.bfloat16
F32 = mybir.dt.float32
NPBF16 = ml_dtypes.bfloat16

NJ = M // 128  # 8 m-chunks
NPAIR = HL // 2  # 2 local head pairs
NCC = C // 128  # 16 key chunks


def _build():
    nc = bacc.Bacc("TRN2", target_bir_lowering=False, debug=False, num_devices=N_CORES)

    xq_d = nc.dram_tensor("xq", [128, NJ, C], BF16, kind="ExternalInput").ap()
    xkv_d = nc.dram_tensor("xkv", [128, NJ, C], BF16, kind="ExternalInput").ap()
    wqp_d = nc.dram_tensor("wqp", [128, NPAIR, NJ, 128], BF16, kind="ExternalInput").ap()
    wkp_d = nc.dram_tensor("wkp", [128, NPAIR, NJ, 128], BF16, kind="ExternalInput").ap()
    wv_d = nc.dram_tensor("wv", [128, NJ, HL * VD], BF16, kind="ExternalInput").ap()
    woa_d = nc.dram_tensor("woa", [64, 8, M], BF16, kind="ExternalInput").ap()
    wob_d = nc.dram_tensor("wob", [64, 8, M], BF16, kind="ExternalInput").ap()
    # mask[v, j] = 1.0 when destination core j is in this core's batch group
    mask_d = nc.dram_tensor("mask", [64, N_CORES], F32, kind="ExternalInput").ap()
    out_d = nc.dram_tensor("out", [DQ, M], F32, kind="ExternalOutput").ap()
    dbg_pre_d = nc.dram_tensor("dbg_pre", [64, HL, C], BF16, kind="ExternalOutput").ap()
    dbg_all_d = nc.dram_tensor("dbg_all", [64, H, DQ], BF16, kind="ExternalOutput").ap()

    with tile.TileContext(nc) as tc:
        with tc.tile_pool(name="sb", bufs=1) as sb, tc.tile_pool(
            name="dram", bufs=1, space="DRAM"
        ) as dram:
            # ---- PE warmup: keep HAM busy while input DMAs land ----------
            warm = sb.tile([128, 128], BF16, name="warm")
            nc.vector.memset(warm, 0.0)
            with tc.tile_pool(name="ps0", space="PSUM", bufs=1) as ps0:
                wps = ps0.tile([128, 512], F32, name="warm_ps")
                for _ in range(20):
                    nc.tensor.matmul(
                        wps[:, 0:128], warm, warm, start=True, stop=True
                    )

            # ---- input loads ---------------------------------------------
            wkp_sb = sb.tile([128, NPAIR, NJ, 128], BF16)
            nc.sync.dma_start(out=wkp_sb, in_=wkp_d)
            wqp_sb = sb.tile([128, NPAIR, NJ, 128], BF16)
            nc.sync.dma_start(out=wqp_sb, in_=wqp_d)
            wv_sb = sb.tile([128, NJ, HL * VD], BF16)
            nc.sync.dma_start(out=wv_sb, in_=wv_d)
            xkv_sb = sb.tile([128, NJ, C], BF16, tag="big_a")
            for j in range(NJ):
                nc.sync.dma_start(out=xkv_sb[:, j, :], in_=xkv_d[:, j, :])
            xq_sb = sb.tile([128, NJ, C], BF16, tag="big_b")
            for j in range(NJ):
                nc.sync.dma_start(out=xq_sb[:, j, :], in_=xq_d[:, j, :])
            wo_sb = [sb.tile([64, 8, M], BF16, tag="big_a", name="woa_sb"),
                     sb.tile([64, 8, M], BF16, tag="big_b", name="wob_sb")]
            nc.sync.dma_start(out=wo_sb[0], in_=woa_d)
            nc.sync.dma_start(out=wo_sb[1], in_=wob_d)

            kt_sb = sb.tile([128, NPAIR, C], BF16)
            qt_sb = sb.tile([128, NPAIR, C], BF16)
            v_sb = sb.tile([128, NCC, HL, VD + 1], BF16)
            nc.vector.memset(v_sb[:, :, :, VD : VD + 1], 1.0)

            # ---- projections ---------------------------------------------
            with tc.tile_pool(name="ps1", space="PSUM", bufs=1) as ps1:
                for p in range(NPAIR):
                    for c4 in range(C // 512):
                        pk = ps1.tile([128, 512], F32, tag="pkq", bufs=3,
                                      name=f"pk_k{p}_{c4}")
                        for j in range(NJ):
                            nc.tensor.matmul(
                                pk, wkp_sb[:, p, j, :],
                                xkv_sb[:, j, c4 * 512 : (c4 + 1) * 512],
                                start=(j == 0), stop=(j == NJ - 1))
                        nc.vector.tensor_copy(
                            kt_sb[:, p, c4 * 512 : (c4 + 1) * 512], pk)
                for p in range(NPAIR):
                    for c4 in range(C // 512):
                        pk = ps1.tile([128, 512], F32, tag="pkq", bufs=3,
                                      name=f"pk_q{p}_{c4}")
                        for j in range(NJ):
                            nc.tensor.matmul(
                                pk, wqp_sb[:, p, j, :],
                                xq_sb[:, j, c4 * 512 : (c4 + 1) * 512],
                                start=(j == 0), stop=(j == NJ - 1))
                        nc.vector.tensor_copy(
                            qt_sb[:, p, c4 * 512 : (c4 + 1) * 512], pk)
                for cc in range(NCC):
                    pv = ps1.tile([128, HL * VD], F32, tag="pv", bufs=2,
                                  name=f"pv_{cc}")
                    for j in range(NJ):
                        nc.tensor.matmul(
                            pv, xkv_sb[:, j, cc * 128 : (cc + 1) * 128],
                            wv_sb[:, j, :],
                            start=(j == 0), stop=(j == NJ - 1))
                    nc.vector.tensor_copy(
                        v_sb[:, cc, :, 0:VD],
                        pv.rearrange("q (h v) -> q h v", h=HL))

            # ---- attention (4 local heads, all 2048 queries) -------------
            pre_sb = sb.tile([64, HL, C], BF16)
            with tc.tile_pool(name="ps2", space="PSUM", bufs=1) as ps2:
                for h in range(HL):
                    p, r = divmod(h, 2)
                    for dh in range(2):  # 1024-query halves
                        pp = ps2.tile([65, 1024], F32, tag="pre", bufs=2,
                                      name=f"pre_{h}_{dh}")
                        for cc in range(NCC):
                            lg = ps2.tile([128, 1024], F32, tag="lg", bufs=2,
                                          name=f"lg_{h}_{dh}_{cc}")
                            for dq in range(2):
                                nc.tensor.matmul(
                                    lg[:, dq * 512 : (dq + 1) * 512],
                                    kt_sb[64 * r : 64 * (r + 1), p,
                                          cc * 128 : (cc + 1) * 128],
                                    qt_sb[64 * r : 64 * (r + 1), p,
                                          dh * 1024 + dq * 512 :
                                          dh * 1024 + (dq + 1) * 512],
                                    start=True, stop=True)
                            att = sb.tile([128, 1024], BF16, tag="att", bufs=3,
                                          name=f"att_{h}_{dh}_{cc}")
                            nc.scalar.activation(
                                att, lg, mybir.ActivationFunctionType.Exp,
                                scale=INV_SCALE)
                            for dq in range(2):
                                nc.tensor.matmul(
                                    pp[:, dq * 512 : (dq + 1) * 512],
                                    v_sb[:, cc, h, :],
                                    att[:, dq * 512 : (dq + 1) * 512],
                                    start=(cc == 0), stop=(cc == NCC - 1))
                        drow = sb.tile([1, 1024], F32, tag="drow", bufs=2,
                                       name=f"drow_{h}_{dh}")
                        nc.vector.tensor_copy(drow, pp[64:65, :])
                        den = sb.tile([64, 1024], F32, tag="den", bufs=2,
                                      name=f"den_{h}_{dh}")
                        nc.gpsimd.partition_broadcast(den, drow)
                        recb = sb.tile([64, 1024], F32, tag="recb", bufs=2,
                                       name=f"recb_{h}_{dh}")
                        nc.vector.reciprocal_approx_fast(recb, den)
                        nc.vector.tensor_mul(
                            pre_sb[:, h, dh * 1024 : (dh + 1) * 1024],
                            pp[0:64, :], recb)

            # ---- AllToAll: head-sharded -> query-sharded -----------------
            # 4-rank groups are unsupported, so run an 8-rank AllToAll where
            # cross-batch destinations receive zeros (host-supplied mask) and
            # the receiver sums the two batch-position blocks.
            mask_sb = sb.tile([64, N_CORES], F32, name="mask_sb")
            nc.sync.dma_start(out=mask_sb, in_=mask_d)
            a2a_in = dram.tile([N_CORES, 64, HL, DQ], BF16, name="a2a_in")
            a2a_out = dram.tile([N_CORES, 64, HL, DQ], BF16, name="a2a_out")
            for g in range(N_CORES):
                stage = sb.tile([64, HL, DQ], BF16, tag="a2a_stage", bufs=2,
                                name=f"stage_{g}")
                nc.vector.tensor_scalar_mul(
                    stage,
                    pre_sb[:, :, (g % GROUP) * DQ : (g % GROUP + 1) * DQ],
                    mask_sb[:, g : g + 1],
                )
                nc.sync.dma_start(out=a2a_in[g], in_=stage)
            nc.gpsimd.collective_compute(
                "AllToAll",
                mybir.AluOpType.bypass,
                ins=[a2a_in.opt()],
                outs=[a2a_out.opt()],
                replica_groups=[list(range(N_CORES))],
            )
            half_b = sb.tile([64, GROUP, HL, DQ], BF16, name="half_b")
            pre_all = sb.tile([64, H, DQ], BF16)
            pre_all_g = pre_all.rearrange("v (g h) d -> v g h d", g=GROUP)
            nc.sync.dma_start(
                out=pre_all_g, in_=a2a_out[0:GROUP].rearrange("g v h d -> v g h d")
            )
            nc.sync.dma_start(
                out=half_b,
                in_=a2a_out[GROUP : 2 * GROUP].rearrange("g v h d -> v g h d"),
            )
            nc.vector.tensor_add(pre_all_g, pre_all_g, half_b)
            nc.sync.dma_start(out=dbg_pre_d, in_=pre_sb)
            nc.sync.dma_start(out=dbg_all_d, in_=pre_all)

            # ---- output projection for the local 512-query slice ---------
            with tc.tile_pool(name="ps3", space="PSUM", bufs=1) as ps3:
                for ds in range(DQ // 128):
                    po = [ps3.tile([128, 512], F32, tag="po", bufs=4,
                                   name=f"po_{ds}_{half}") for half in range(2)]
                    for h in range(H):
                        for half in range(2):
                            nc.tensor.matmul(
                                po[half],
                                pre_all[:, h, ds * 128 : (ds + 1) * 128],
                                wo_sb[h // 8][:, h % 8,
                                              half * 512 : (half + 1) * 512],
                                start=(h == 0), stop=(h == H - 1))
                    osb = sb.tile([128, M], F32, tag="osb", bufs=2,
                                  name=f"osb_{ds}")
                    nc.vector.tensor_copy(osb[:, 0:512], po[0])
                    nc.vector.tensor_copy(osb[:, 512:1024], po[1])
                    nc.sync.dma_start(
                        out=out_d[ds * 128 : (ds + 1) * 128, :], in_=osb)

    nc.compile()
    return nc


_CACHE: dict = {}


def _prep_inputs(kvinput, qinput, wq, wk, wv, wo):
    """Host-side sharding/layout prep. Returns per-core input dicts."""

    def bf16c(a):
        return np.ascontiguousarray(a.astype(NPBF16))

    def pack_pairs(w):  # [2*npair, M, 64] -> [128(q), npair, NJ, 128]
        npair = w.shape[0] // 2
        a = w.reshape(npair, 2, M, KD).transpose(0, 2, 1, 3).reshape(npair, M, 128)
        return bf16c(a.reshape(npair, NJ, 128, 128).transpose(2, 0, 1, 3))

    wq = np.asarray(wq, np.float32)
    wk = np.asarray(wk, np.float32)
    wv = np.asarray(wv, np.float32)
    wof = np.asarray(wo, np.float32).transpose(1, 0, 2)  # [64, H, M]
    woa = bf16c(wof[:, 0:8, :])
    wob = bf16c(wof[:, 8:16, :])

    kvinput = np.asarray(kvinput, np.float32)
    qinput = np.asarray(qinput, np.float32)

    def xpose(x):  # [C, M] -> [128, NJ, C]
        return bf16c(x.T.reshape(NJ, 128, C).transpose(1, 0, 2))

    xkv_b = [xpose(kvinput[b]) for b in range(B)]
    xq_b = [xpose(qinput[b]) for b in range(B)]

    in_maps = []
    for core in range(N_CORES):
        b, r = divmod(core, GROUP)
        hsl = slice(r * HL, (r + 1) * HL)
        mask = np.zeros((64, N_CORES), np.float32)
        mask[:, b * GROUP : (b + 1) * GROUP] = 1.0
        in_maps.append(
            {
                "mask": mask,
                "xq": xq_b[b],
                "xkv": xkv_b[b],
                "wqp": pack_pairs(wq[hsl]),
                "wkp": pack_pairs(wk[hsl]),
                "wv": bf16c(
                    wv[hsl]
                    .transpose(1, 0, 2)
                    .reshape(NJ, 128, HL * VD)
                    .transpose(1, 0, 2)
                ),
                "woa": woa,
                "wob": wob,
            }
        )
    return in_maps


def kernel(**inputs):
    if "nc" not in _CACHE:
        _CACHE["nc"] = _build()
    nc = _CACHE["nc"]
    in_maps = _prep_inputs(
        inputs["kvinput"],
        inputs["qinput"],
        inputs["wq"],
        inputs["wk"],
        inputs["wv"],
        inputs["wo"],
    )
    _CACHE["in_maps"] = in_maps
    res = run_bass_kernel_spmd(nc, in_maps, core_ids=list(range(N_CORES)))
    out = np.empty((B, C, M), np.float32)
    for core in range(N_CORES):
        b, r = divmod(core, GROUP)
        out[b, r * DQ : (r + 1) * DQ, :] = res.results[core]["out"]
    return out


# revision 15
# speedup vs baseline: 1.1498x; 1.0584x over previous
"""Multi-head attention (B=2, C=2048, M=1024, H=16, K=V=64) on 8 TRN2 NeuronCores.

Sharding (per the tensor-parallel hint, with a cheaper exchange): core i handles
batch b = i//4 and heads 4r..4r+3 where r = i%4. Each core projects K/V/Q for
its 4 heads, runs attention for those heads over all 2048 queries, then an
AllToAll within each 4-core batch group redistributes the normalized attention
output "pre" [v, h, d] (1 MB bf16 per core) from head-sharded to query-sharded,
so every core computes the full output projection for its 512-query slice with
no reduction.

All matmuls are bf16 with fp32 PSUM accumulation. Softmax runs along the PSUM
partition axis (keys on partitions): exp via ScalarE on wide [128, 1024] tiles,
denominator via a ones-column folded into the att@V matmul, normalization via
fast approximate reciprocal + gpsimd partition-broadcast on the small pre
tensor. No max-subtraction is needed: scaled logits are ~N(0,1) so exp stays
in fp32 range.
"""

import os
import sys

import numpy as np
import ml_dtypes

for _p in ("/opt/trn_rl_repo", os.path.expanduser("~/.axon_site/_ro/trn_rl_repo")):
    if os.path.isdir(_p) and _p not in sys.path:
        sys.path.insert(0, _p)

import concourse.bass as bass
import concourse.bacc as bacc
import concourse.tile as tile
from concourse import mybir
from concourse.bass_utils import run_bass_kernel_spmd

B, C, M, H, KD, VD = 2, 2048, 1024, 16, 64, 64
N_CORES = 8
GROUP = 4  # cores per batch group
HL = H // GROUP  # 4 local heads per core
DQ = C // GROUP  # 512-query output slice per core
INV_SCALE = 1.0 / 8.0  # 1/sqrt(KD)
BF16 = mybir.dt.bfloat16
F32 = mybir.dt.float32
NPBF16 = ml_dtypes.bfloat16

NJ = M // 128  # 8 m-chunks
NPAIR = HL // 2  # 2 local head pairs
NCC = C // 128  # 16 key chunks


def _build():
    nc = bacc.Bacc("TRN2", target_bir_lowering=False, debug=False, num_devices=N_CORES)

    xq_d = nc.dram_tensor("xq", [128, NJ, C], BF16, kind="ExternalInput").ap()
    xkv_d = nc.dram_tensor("xkv", [128, NJ, C], BF16, kind="ExternalInput").ap()
    wqp_d = nc.dram_tensor("wqp", [128, NPAIR, NJ, 128], BF16, kind="ExternalInput").ap()
    wkp_d = nc.dram_tensor("wkp", [128, NPAIR, NJ, 128], BF16, kind="ExternalInput").ap()
    wv_d = nc.dram_tensor("wv", [128, NJ, HL * VD], BF16, kind="ExternalInput").ap()
    woa_d = nc.dram_tensor("woa", [64, 8, M], BF16, kind="ExternalInput").ap()
    wob_d = nc.dram_tensor("wob", [64, 8, M], BF16, kind="ExternalInput").ap()
    # ridx[g] = flat source index (rank*GROUP + my_dslice) for receive DMAs
    ridx_d = nc.dram_tensor("ridx", [1, GROUP], mybir.dt.uint32, kind="ExternalInput").ap()
    out_d = nc.dram_tensor("out", [DQ, M], F32, kind="ExternalOutput").ap()
    dbg_pre_d = nc.dram_tensor("dbg_pre", [64, HL, C], BF16, kind="ExternalOutput").ap()
    dbg_all_d = nc.dram_tensor("dbg_all", [64, H, DQ], BF16, kind="ExternalOutput").ap()

    with tile.TileContext(nc) as tc:
        with tc.tile_pool(name="sb", bufs=1) as sb, tc.tile_pool(
            name="dram", bufs=1, space="DRAM"
        ) as dram:
            # ---- PE warmup: keep HAM busy while input DMAs land ----------
            warm = sb.tile([128, 128], BF16, name="warm")
            nc.vector.memset(warm, 0.0)
            with tc.tile_pool(name="ps0", space="PSUM", bufs=1) as ps0:
                wps = ps0.tile([128, 512], F32, name="warm_ps")
                for _ in range(20):
                    nc.tensor.matmul(
                        wps[:, 0:128], warm, warm, start=True, stop=True
                    )

            # ---- input loads ---------------------------------------------
            wkp_sb = sb.tile([128, NPAIR, NJ, 128], BF16)
            nc.sync.dma_start(out=wkp_sb, in_=wkp_d)
            wqp_sb = sb.tile([128, NPAIR, NJ, 128], BF16)
            nc.sync.dma_start(out=wqp_sb, in_=wqp_d)
            wv_sb = sb.tile([128, NJ, HL * VD], BF16)
            nc.sync.dma_start(out=wv_sb, in_=wv_d)
            xkv_sb = sb.tile([128, NJ, C], BF16, tag="big_a")
            for j in range(NJ):
                nc.sync.dma_start(out=xkv_sb[:, j, :], in_=xkv_d[:, j, :])
            xq_sb = sb.tile([128, NJ, C], BF16, tag="big_b")
            for j in range(NJ):
                nc.sync.dma_start(out=xq_sb[:, j, :], in_=xq_d[:, j, :])
            wo_sb = [sb.tile([64, 8, M], BF16, tag="big_a", name="woa_sb"),
                     sb.tile([64, 8, M], BF16, tag="big_b", name="wob_sb")]
            nc.sync.dma_start(out=wo_sb[0], in_=woa_d)
            nc.sync.dma_start(out=wo_sb[1], in_=wob_d)

            kt_sb = sb.tile([128, NPAIR, C], BF16)
            qt_sb = sb.tile([128, NPAIR, C], BF16)
            v_sb = sb.tile([128, NCC, HL, VD + 1], BF16)
            nc.vector.memset(v_sb[:, :, :, VD : VD + 1], 1.0)

            # ---- projections ---------------------------------------------
            with tc.tile_pool(name="ps1", space="PSUM", bufs=1) as ps1:
                for p in range(NPAIR):
                    for c4 in range(C // 512):
                        pk = ps1.tile([128, 512], F32, tag="pkq", bufs=3,
                                      name=f"pk_k{p}_{c4}")
                        for j in range(NJ):
                            nc.tensor.matmul(
                                pk, wkp_sb[:, p, j, :],
                                xkv_sb[:, j, c4 * 512 : (c4 + 1) * 512],
                                start=(j == 0), stop=(j == NJ - 1))
                        nc.vector.tensor_copy(
                            kt_sb[:, p, c4 * 512 : (c4 + 1) * 512], pk)
                for p in range(NPAIR):
                    for c4 in range(C // 512):
                        pk = ps1.tile([128, 512], F32, tag="pkq", bufs=3,
                                      name=f"pk_q{p}_{c4}")
                        for j in range(NJ):
                            nc.tensor.matmul(
                                pk, wqp_sb[:, p, j, :],
                                xq_sb[:, j, c4 * 512 : (c4 + 1) * 512],
                                start=(j == 0), stop=(j == NJ - 1))
                        nc.vector.tensor_copy(
                            qt_sb[:, p, c4 * 512 : (c4 + 1) * 512], pk)
                for cc in range(NCC):
                    pv = ps1.tile([128, HL * VD], F32, tag="pv", bufs=2,
                                  name=f"pv_{cc}")
                    for j in range(NJ):
                        nc.tensor.matmul(
                            pv, xkv_sb[:, j, cc * 128 : (cc + 1) * 128],
                            wv_sb[:, j, :],
                            start=(j == 0), stop=(j == NJ - 1))
                    nc.vector.tensor_copy(
                        v_sb[:, cc, :, 0:VD],
                        pv.rearrange("q (h v) -> q h v", h=HL))

            # ---- attention (4 local heads, all 2048 queries) -------------
            pre_sb = sb.tile([64, HL, C], BF16)
            with tc.tile_pool(name="ps2", space="PSUM", bufs=1) as ps2:
                for h in range(HL):
                    p, r = divmod(h, 2)
                    for dh in range(2):  # 1024-query halves
                        pp = ps2.tile([65, 1024], F32, tag="pre", bufs=2,
                                      name=f"pre_{h}_{dh}")
                        for cc in range(NCC):
                            lg = ps2.tile([128, 1024], F32, tag="lg", bufs=2,
                                          name=f"lg_{h}_{dh}_{cc}")
                            for dq in range(2):
                                nc.tensor.matmul(
                                    lg[:, dq * 512 : (dq + 1) * 512],
                                    kt_sb[64 * r : 64 * (r + 1), p,
                                          cc * 128 : (cc + 1) * 128],
                                    qt_sb[64 * r : 64 * (r + 1), p,
                                          dh * 1024 + dq * 512 :
                                          dh * 1024 + (dq + 1) * 512],
                                    start=True, stop=True)
                            att = sb.tile([128, 1024], BF16, tag="att", bufs=3,
                                          name=f"att_{h}_{dh}_{cc}")
                            nc.scalar.activation(
                                att, lg, mybir.ActivationFunctionType.Exp,
                                scale=INV_SCALE)
                            for dq in range(2):
                                nc.tensor.matmul(
                                    pp[:, dq * 512 : (dq + 1) * 512],
                                    v_sb[:, cc, h, :],
                                    att[:, dq * 512 : (dq + 1) * 512],
                                    start=(cc == 0), stop=(cc == NCC - 1))
                        drow = sb.tile([1, 1024], F32, tag="drow", bufs=2,
                                       name=f"drow_{h}_{dh}")
                        nc.vector.tensor_copy(drow, pp[64:65, :])
                        den = sb.tile([64, 1024], F32, tag="den", bufs=2,
                                      name=f"den_{h}_{dh}")
                        nc.gpsimd.partition_broadcast(den, drow)
                        recb = sb.tile([64, 1024], F32, tag="recb", bufs=2,
                                       name=f"recb_{h}_{dh}")
                        nc.vector.reciprocal_approx_fast(recb, den)
                        nc.vector.tensor_mul(
                            pre_sb[:, h, dh * 1024 : (dh + 1) * 1024],
                            pp[0:64, :], recb)

            # ---- exchange: head-sharded -> query-sharded -----------------
            # 8-rank mesh AllGather of the local pre (1 MB/rank); each core
            # then pulls its batch group's 4 source-rank slices for its own
            # 512-query range via register-indexed (host-supplied) DMAs.
            ag_in = dram.tile([GROUP, 64, HL, DQ], BF16, name="ag_in")
            ag_out = dram.tile([N_CORES, GROUP, 64, HL, DQ], BF16, name="ag_out",
                               addr_space="Shared")
            for g in range(GROUP):
                nc.sync.dma_start(
                    out=ag_in[g], in_=pre_sb[:, :, g * DQ : (g + 1) * DQ])
            nc.gpsimd.collective_compute(
                "AllGather",
                mybir.AluOpType.bypass,
                ins=[ag_in.opt()],
                outs=[ag_out.opt()],
                replica_groups=[[list(range(N_CORES))][0]],
            )
            ridx_sb = sb.tile([1, GROUP], mybir.dt.uint32, name="ridx_sb")
            nc.sync.dma_start(out=ridx_sb, in_=ridx_d)
            v32 = ag_out.rearrange("g s v h d -> (g s) v h d")
            pre_all = sb.tile([64, H, DQ], BF16)
            for gp in range(GROUP):
                reg = nc.alloc_registers(f"ridx_{gp}")
                nc.regs_load(reg, ridx_sb[0:1, gp : gp + 1])
                rv = nc.snap(reg, donate=True, min_val=0,
                             max_val=N_CORES * GROUP - 1)
                nc.sync.dma_start(
                    out=pre_all[:, gp * HL : (gp + 1) * HL, :],
                    in_=v32[bass.ds(rv, 1), :, :, :].rearrange(
                        "o v h d -> v (o h) d"
                    ),
                )

            # ---- output projection for the local 512-query slice ---------
            with tc.tile_pool(name="ps3", space="PSUM", bufs=1) as ps3:
                for ds in range(DQ // 128):
                    po = [ps3.tile([128, 512], F32, tag="po", bufs=4,
                                   name=f"po_{ds}_{half}") for half in range(2)]
                    for h in range(H):
                        for half in range(2):
                            nc.tensor.matmul(
                                po[half],
                                pre_all[:, h, ds * 128 : (ds + 1) * 128],
                                wo_sb[h // 8][:, h % 8,
                                              half * 512 : (half + 1) * 512],
                                start=(h == 0), stop=(h == H - 1))
                    osb = sb.tile([128, M], F32, tag="osb", bufs=2,
                                  name=f"osb_{ds}")
                    nc.vector.tensor_copy(osb[:, 0:512], po[0])
                    nc.vector.tensor_copy(osb[:, 512:1024], po[1])
                    nc.sync.dma_start(
                        out=out_d[ds * 128 : (ds + 1) * 128, :], in_=osb)

    nc.compile()
    return nc


_CACHE: dict = {}


def _prep_inputs(kvinput, qinput, wq, wk, wv, wo):
    """Host-side sharding/layout prep. Returns per-core input dicts."""

    def bf16c(a):
        return np.ascontiguousarray(a.astype(NPBF16))

    def pack_pairs(w):  # [2*npair, M, 64] -> [128(q), npair, NJ, 128]
        npair = w.shape[0] // 2
        a = w.reshape(npair, 2, M, KD).transpose(0, 2, 1, 3).reshape(npair, M, 128)
        return bf16c(a.reshape(npair, NJ, 128, 128).transpose(2, 0, 1, 3))

    wq = np.asarray(wq, np.float32)
    wk = np.asarray(wk, np.float32)
    wv = np.asarray(wv, np.float32)
    wof = np.asarray(wo, np.float32).transpose(1, 0, 2)  # [64, H, M]
    woa = bf16c(wof[:, 0:8, :])
    wob = bf16c(wof[:, 8:16, :])

    kvinput = np.asarray(kvinput, np.float32)
    qinput = np.asarray(qinput, np.float32)

    def xpose(x):  # [C, M] -> [128, NJ, C]
        return bf16c(x.T.reshape(NJ, 128, C).transpose(1, 0, 2))

    xkv_b = [xpose(kvinput[b]) for b in range(B)]
    xq_b = [xpose(qinput[b]) for b in range(B)]

    in_maps = []
    for core in range(N_CORES):
        b, r = divmod(core, GROUP)
        hsl = slice(r * HL, (r + 1) * HL)
        ridx = np.array(
            [[(b * GROUP + g) * GROUP + r for g in range(GROUP)]], np.uint32
        )
        in_maps.append(
            {
                "ridx": ridx,
                "xq": xq_b[b],
                "xkv": xkv_b[b],
                "wqp": pack_pairs(wq[hsl]),
                "wkp": pack_pairs(wk[hsl]),
                "wv": bf16c(
                    wv[hsl]
                    .transpose(1, 0, 2)
                    .reshape(NJ, 128, HL * VD)
                    .transpose(1, 0, 2)
                ),
                "woa": woa,
                "wob": wob,
            }
        )
    return in_maps


def kernel(**inputs):
    if "nc" not in _CACHE:
        _CACHE["nc"] = _build()
    nc = _CACHE["nc"]
    in_maps = _prep_inputs(
        inputs["kvinput"],
        inputs["qinput"],
        inputs["wq"],
        inputs["wk"],
        inputs["wv"],
        inputs["wo"],
    )
    _CACHE["in_maps"] = in_maps
    res = run_bass_kernel_spmd(nc, in_maps, core_ids=list(range(N_CORES)))
    out = np.empty((B, C, M), np.float32)
    for core in range(N_CORES):
        b, r = divmod(core, GROUP)
        out[b, r * DQ : (r + 1) * DQ, :] = res.results[core]["out"]
    return out


# revision 18
# speedup vs baseline: 1.2435x; 1.0815x over previous
"""Multi-head attention (B=2, C=2048, M=1024, H=16, K=V=64) on 8 TRN2 NeuronCores.

Sharding (per the tensor-parallel hint, with a cheaper exchange): core i handles
batch b = i//4 and heads 4r..4r+3 where r = i%4. Each core projects K/V/Q for
its 4 heads, runs attention for those heads over all 2048 queries, then an
AllToAll within each 4-core batch group redistributes the normalized attention
output "pre" [v, h, d] (1 MB bf16 per core) from head-sharded to query-sharded,
so every core computes the full output projection for its 512-query slice with
no reduction.

All matmuls are bf16 with fp32 PSUM accumulation. Softmax runs along the PSUM
partition axis (keys on partitions): exp via ScalarE on wide [128, 1024] tiles,
denominator via a ones-column folded into the att@V matmul, normalization via
fast approximate reciprocal + gpsimd partition-broadcast on the small pre
tensor. No max-subtraction is needed: scaled logits are ~N(0,1) so exp stays
in fp32 range.
"""

import os
import sys

import numpy as np
import ml_dtypes

for _p in ("/opt/trn_rl_repo", os.path.expanduser("~/.axon_site/_ro/trn_rl_repo")):
    if os.path.isdir(_p) and _p not in sys.path:
        sys.path.insert(0, _p)

import concourse.bass as bass
import concourse.bacc as bacc
import concourse.tile as tile
from concourse import mybir
from concourse.bass_utils import run_bass_kernel_spmd

B, C, M, H, KD, VD = 2, 2048, 1024, 16, 64, 64
N_CORES = 8
GROUP = 4  # cores per batch group
HL = H // GROUP  # 4 local heads per core
DQ = C // GROUP  # 512-query output slice per core
INV_SCALE = 1.0 / 8.0  # 1/sqrt(KD)
BF16 = mybir.dt.bfloat16
F32 = mybir.dt.float32
NPBF16 = ml_dtypes.bfloat16

NJ = M // 128  # 8 m-chunks
NPAIR = HL // 2  # 2 local head pairs
NCC = C // 128  # 16 key chunks


def _build():
    nc = bacc.Bacc("TRN2", target_bir_lowering=False, debug=False, num_devices=N_CORES)

    xq_d = nc.dram_tensor("xq", [128, NJ, C], BF16, kind="ExternalInput").ap()
    xkv_d = nc.dram_tensor("xkv", [128, NJ, C], BF16, kind="ExternalInput").ap()
    wqp_d = nc.dram_tensor("wqp", [128, NPAIR, NJ, 128], BF16, kind="ExternalInput").ap()
    wkp_d = nc.dram_tensor("wkp", [128, NPAIR, NJ, 128], BF16, kind="ExternalInput").ap()
    wv_d = nc.dram_tensor("wv", [128, NJ, HL * VD], BF16, kind="ExternalInput").ap()
    woa_d = nc.dram_tensor("woa", [64, 8, M], BF16, kind="ExternalInput").ap()
    wob_d = nc.dram_tensor("wob", [64, 8, M], BF16, kind="ExternalInput").ap()
    # ridx[g] = flat source index (rank*GROUP + my_dslice) for receive DMAs
    ridx_d = nc.dram_tensor("ridx", [1, GROUP], mybir.dt.uint32, kind="ExternalInput").ap()
    out_d = nc.dram_tensor("out", [DQ, M], F32, kind="ExternalOutput").ap()
    dbg_pre_d = nc.dram_tensor("dbg_pre", [64, HL, C], BF16, kind="ExternalOutput").ap()
    dbg_all_d = nc.dram_tensor("dbg_all", [64, H, DQ], BF16, kind="ExternalOutput").ap()

    with tile.TileContext(nc) as tc:
        with tc.tile_pool(name="sb", bufs=1) as sb, tc.tile_pool(
            name="dram", bufs=1, space="DRAM"
        ) as dram:
            # ---- PE warmup: keep HAM busy while input DMAs land ----------
            warm = sb.tile([128, 128], BF16, name="warm")
            nc.vector.memset(warm, 0.0)
            with tc.tile_pool(name="ps0", space="PSUM", bufs=1) as ps0:
                wps = ps0.tile([128, 512], F32, name="warm_ps")
                for _ in range(20):
                    nc.tensor.matmul(
                        wps[:, 0:128], warm, warm, start=True, stop=True
                    )

            # ---- input loads ---------------------------------------------
            wkp_sb = sb.tile([128, NPAIR, NJ, 128], BF16)
            nc.sync.dma_start(out=wkp_sb, in_=wkp_d)
            wqp_sb = sb.tile([128, NPAIR, NJ, 128], BF16)
            nc.sync.dma_start(out=wqp_sb, in_=wqp_d)
            wv_sb = sb.tile([128, NJ, HL * VD], BF16)
            nc.sync.dma_start(out=wv_sb, in_=wv_d)
            xkv_sb = sb.tile([128, NJ, C], BF16, tag="big_a")
            for j in range(NJ):
                nc.sync.dma_start(out=xkv_sb[:, j, :], in_=xkv_d[:, j, :])
            xq_sb = sb.tile([128, NJ, C], BF16, tag="big_b")
            for j in range(NJ):
                nc.sync.dma_start(out=xq_sb[:, j, :], in_=xq_d[:, j, :])
            wo_sb = [sb.tile([64, 8, M], BF16, tag="big_a", name="woa_sb"),
                     sb.tile([64, 8, M], BF16, tag="big_b", name="wob_sb")]
            nc.sync.dma_start(out=wo_sb[0], in_=woa_d)
            nc.sync.dma_start(out=wo_sb[1], in_=wob_d)

            # per-d-half exchange buffers; ag_all[dh] = [8rank, 2ds, 64, HL, 512]
            ag_all = dram.tile([2, N_CORES, 2, 64, HL, DQ], BF16, name="ag_all")
            ag_in = dram.tile([2, 2, 64, HL, DQ], BF16, name="ag_in")

            kt_sb = sb.tile([128, NPAIR, C], BF16)
            qt_sb = sb.tile([128, NPAIR, C], BF16)
            v_sb = sb.tile([128, NCC, HL, VD + 1], BF16)
            nc.vector.memset(v_sb[:, :, :, VD : VD + 1], 1.0)

            # ---- projections ---------------------------------------------
            with tc.tile_pool(name="ps1", space="PSUM", bufs=1) as ps1:
                for p in range(NPAIR):
                    for c4 in range(C // 512):
                        pk = ps1.tile([128, 512], F32, tag="pkq", bufs=3,
                                      name=f"pk_k{p}_{c4}")
                        for j in range(NJ):
                            nc.tensor.matmul(
                                pk, wkp_sb[:, p, j, :],
                                xkv_sb[:, j, c4 * 512 : (c4 + 1) * 512],
                                start=(j == 0), stop=(j == NJ - 1))
                        nc.vector.tensor_copy(
                            kt_sb[:, p, c4 * 512 : (c4 + 1) * 512], pk)
                for p in range(NPAIR):
                    for c4 in range(C // 512):
                        pk = ps1.tile([128, 512], F32, tag="pkq", bufs=3,
                                      name=f"pk_q{p}_{c4}")
                        for j in range(NJ):
                            nc.tensor.matmul(
                                pk, wqp_sb[:, p, j, :],
                                xq_sb[:, j, c4 * 512 : (c4 + 1) * 512],
                                start=(j == 0), stop=(j == NJ - 1))
                        nc.vector.tensor_copy(
                            qt_sb[:, p, c4 * 512 : (c4 + 1) * 512], pk)
                for cc in range(NCC):
                    pv = ps1.tile([128, HL * VD], F32, tag="pv", bufs=2,
                                  name=f"pv_{cc}")
                    for j in range(NJ):
                        nc.tensor.matmul(
                            pv, xkv_sb[:, j, cc * 128 : (cc + 1) * 128],
                            wv_sb[:, j, :],
                            start=(j == 0), stop=(j == NJ - 1))
                    nc.vector.tensor_copy(
                        v_sb[:, cc, :, 0:VD],
                        pv.rearrange("q (h v) -> q h v", h=HL))

            # ---- attention (4 local heads, all 2048 queries) -------------
            # d-halves outer so the first half's exchange overlaps the second
            # half's attention.
            pre_sb = sb.tile([64, HL, C], BF16)
            with tc.tile_pool(name="ps2", space="PSUM", bufs=1) as ps2:
                for dh in range(2):
                    for h in range(HL):
                        p, r = divmod(h, 2)
                        pp = ps2.tile([65, 1024], F32, tag="pre", bufs=2,
                                      name=f"pre_{h}_{dh}")
                        for cc in range(NCC):
                            lg = ps2.tile([128, 1024], F32, tag="lg", bufs=2,
                                          name=f"lg_{h}_{dh}_{cc}")
                            for dq in range(2):
                                nc.tensor.matmul(
                                    lg[:, dq * 512 : (dq + 1) * 512],
                                    kt_sb[64 * r : 64 * (r + 1), p,
                                          cc * 128 : (cc + 1) * 128],
                                    qt_sb[64 * r : 64 * (r + 1), p,
                                          dh * 1024 + dq * 512 :
                                          dh * 1024 + (dq + 1) * 512],
                                    start=True, stop=True)
                            att = sb.tile([128, 1024], BF16, tag="att", bufs=3,
                                          name=f"att_{h}_{dh}_{cc}")
                            nc.scalar.activation(
                                att, lg, mybir.ActivationFunctionType.Exp,
                                scale=INV_SCALE)
                            for dq in range(2):
                                nc.tensor.matmul(
                                    pp[:, dq * 512 : (dq + 1) * 512],
                                    v_sb[:, cc, h, :],
                                    att[:, dq * 512 : (dq + 1) * 512],
                                    start=(cc == 0), stop=(cc == NCC - 1))
                        drow = sb.tile([1, 1024], F32, tag="drow", bufs=2,
                                       name=f"drow_{h}_{dh}")
                        nc.vector.tensor_copy(drow, pp[64:65, :])
                        den = sb.tile([64, 1024], F32, tag="den", bufs=2,
                                      name=f"den_{h}_{dh}")
                        nc.gpsimd.partition_broadcast(den, drow)
                        recb = sb.tile([64, 1024], F32, tag="recb", bufs=2,
                                       name=f"recb_{h}_{dh}")
                        nc.vector.reciprocal_approx_fast(recb, den)
                        nc.vector.tensor_mul(
                            pre_sb[:, h, dh * 1024 : (dh + 1) * 1024],
                            pp[0:64, :], recb)
                    # stage + launch this d-half's AllGather immediately
                    for gh in range(2):
                        g = dh * 2 + gh
                        nc.sync.dma_start(
                            out=ag_in[dh, gh],
                            in_=pre_sb[:, :, g * DQ : (g + 1) * DQ])
                    nc.gpsimd.collective_compute(
                        "AllGather",
                        mybir.AluOpType.bypass,
                        ins=[ag_in[dh].opt()],
                        outs=[ag_all[dh].opt()],
                        replica_groups=[list(range(N_CORES))],
                    )

            # ---- receive: pull my 4 source slices via host-indexed DMAs --
            ridx_sb = sb.tile([1, GROUP], mybir.dt.uint32, name="ridx_sb")
            nc.sync.dma_start(out=ridx_sb, in_=ridx_d)
            v32 = ag_all.rearrange("x g s v h d -> (x g s) v h d")
            pre_all = sb.tile([64, H, DQ], BF16)
            for gp in range(GROUP):
                reg = nc.alloc_registers(f"ridx_{gp}")
                nc.regs_load(reg, ridx_sb[0:1, gp : gp + 1])
                rv = nc.snap(reg, donate=True, min_val=0,
                             max_val=2 * N_CORES * 2 - 1)
                nc.sync.dma_start(
                    out=pre_all[:, gp * HL : (gp + 1) * HL, :],
                    in_=v32[bass.ds(rv, 1), :, :, :].rearrange(
                        "o v h d -> v (o h) d"
                    ),
                )

            # ---- output projection for the local 512-query slice ---------
            with tc.tile_pool(name="ps3", space="PSUM", bufs=1) as ps3:
                for ds in range(DQ // 128):
                    po = [ps3.tile([128, 512], F32, tag="po", bufs=4,
                                   name=f"po_{ds}_{half}") for half in range(2)]
                    for h in range(H):
                        for half in range(2):
                            nc.tensor.matmul(
                                po[half],
                                pre_all[:, h, ds * 128 : (ds + 1) * 128],
                                wo_sb[h // 8][:, h % 8,
                                              half * 512 : (half + 1) * 512],
                                start=(h == 0), stop=(h == H - 1))
                    osb = sb.tile([128, M], F32, tag="osb", bufs=2,
                                  name=f"osb_{ds}")
                    nc.vector.tensor_copy(osb[:, 0:512], po[0])
                    nc.vector.tensor_copy(osb[:, 512:1024], po[1])
                    nc.sync.dma_start(
                        out=out_d[ds * 128 : (ds + 1) * 128, :], in_=osb)

    nc.compile()
    return nc


_CACHE: dict = {}


def _prep_inputs(kvinput, qinput, wq, wk, wv, wo):
    """Host-side sharding/layout prep. Returns per-core input dicts."""

    def bf16c(a):
        return np.ascontiguousarray(a.astype(NPBF16))

    def pack_pairs(w):  # [2*npair, M, 64] -> [128(q), npair, NJ, 128]
        npair = w.shape[0] // 2
        a = w.reshape(npair, 2, M, KD).transpose(0, 2, 1, 3).reshape(npair, M, 128)
        return bf16c(a.reshape(npair, NJ, 128, 128).transpose(2, 0, 1, 3))

    wq = np.asarray(wq, np.float32)
    wk = np.asarray(wk, np.float32)
    wv = np.asarray(wv, np.float32)
    wof = np.asarray(wo, np.float32).transpose(1, 0, 2)  # [64, H, M]
    woa = bf16c(wof[:, 0:8, :])
    wob = bf16c(wof[:, 8:16, :])

    kvinput = np.asarray(kvinput, np.float32)
    qinput = np.asarray(qinput, np.float32)

    def xpose(x):  # [C, M] -> [128, NJ, C]
        return bf16c(x.T.reshape(NJ, 128, C).transpose(1, 0, 2))

    xkv_b = [xpose(kvinput[b]) for b in range(B)]
    xq_b = [xpose(qinput[b]) for b in range(B)]

    in_maps = []
    for core in range(N_CORES):
        b, r = divmod(core, GROUP)
        hsl = slice(r * HL, (r + 1) * HL)
        ridx = np.array(
            [[(r // 2) * N_CORES * 2 + (b * GROUP + g) * 2 + (r % 2)
              for g in range(GROUP)]], np.uint32
        )
        in_maps.append(
            {
                "ridx": ridx,
                "xq": xq_b[b],
                "xkv": xkv_b[b],
                "wqp": pack_pairs(wq[hsl]),
                "wkp": pack_pairs(wk[hsl]),
                "wv": bf16c(
                    wv[hsl]
                    .transpose(1, 0, 2)
                    .reshape(NJ, 128, HL * VD)
                    .transpose(1, 0, 2)
                ),
                "woa": woa,
                "wob": wob,
            }
        )
    return in_maps


def kernel(**inputs):
    if "nc" not in _CACHE:
        _CACHE["nc"] = _build()
    nc = _CACHE["nc"]
    in_maps = _prep_inputs(
        inputs["kvinput"],
        inputs["qinput"],
        inputs["wq"],
        inputs["wk"],
        inputs["wv"],
        inputs["wo"],
    )
    _CACHE["in_maps"] = in_maps
    res = run_bass_kernel_spmd(nc, in_maps, core_ids=list(range(N_CORES)))
    out = np.empty((B, C, M), np.float32)
    for core in range(N_CORES):
        b, r = divmod(core, GROUP)
        out[b, r * DQ : (r + 1) * DQ, :] = res.results[core]["out"]
    return out
